# revision 41
# baseline (speedup 1.0000x reference)
"""Additive-attention kernel for TRN2, data-parallel over batch across 8 NeuronCores.

Reference computation (per batch b):
    energy[t,h] = tanh( enc[t,:] @ We[h,:] + hidden[b,:] @ Wh[h,:] + b_attn[h] )
    scores[t]   = energy[t,:] @ v
    out[b,0,:]  = softmax(scores)

Shapes: B=32, T=2048, D=1024, H=512.  W_attn = [Wh | We] : [H, 2D].

Per-core (4 batches): dominant work is enc @ We^T (8.6 GFLOP, 33.5 MB f32 HBM
traffic) -> ridge regime: bf16 TensorE floor ~110us, HBM floor ~94us.

Engine plan per t-tile (512 t-rows, 16 tiles per core):
  - sync HWDGE: f32 enc loads (4 per-q chunks), prefetched 4 tiles ahead.
  - DVE: f32 -> bf16 casts; scores reduction tree (energy*v, summed over the
    4 h-chunks) so the per-(t) score needs only ONE 128-partition ones-matmul
    on PE instead of four v-matmuls.
  - ACT HWDGE: XBAR dma_start_transpose of the bf16 tile into encT (d on
    partitions) - takes all 32 128x128 transposes (and their PSUM drains)
    off TensorE.
  - TensorE: only the main matmul remains: psum[h=128,t=512] += WeT[dc]
    (stationary) @ encT[dc] x8, emitted ht-inner so consecutive matmuls hit
    different PSUM banks (the accumulate RMW turnaround of one bank hides
    under the next bank's streaming: 216ns/MM vs 259 for dc-inner), plus one
    ones-matmul for the partition-sum of the score tree.
  - ACT: energy = tanh(psum + c[b,h]) fused per-partition bias; exp of the
    scores row fused into the psum->sbuf copy with accum_out partial sums
    (softmax max-subtraction dropped: |scores| <= sum|v| ~ 18, exp is safe
    in f32), so the tail is just reduce+recip+normalize+DMA for all batches.
"""

import numpy as np
import ml_dtypes

import concourse.bass as bass
import concourse.mybir as mybir
import concourse.tile as tile
from concourse import bacc
from concourse.bass_utils import run_bass_kernel_spmd

B, T, D, H = 32, 2048, 1024, 512
NCORES = 8
BC = B // NCORES          # batches per core
TT = 512                  # t-tile (psum free dim)
NTT = T // TT             # 4 t-tiles per batch
DC = D // 128             # 8 contraction chunks
HT = H // 128             # 4 h tiles

F32 = mybir.dt.float32
BF16 = mybir.dt.bfloat16

_BUILD_CACHE = {}


def _build_nc():
    """Build the SPMD Bass graph (same on all 8 cores)."""
    nc = bacc.Bacc("TRN2", target_bir_lowering=False, debug=False,
                   num_devices=NCORES)

    enc = nc.dram_tensor("enc", [BC, T, D], F32, kind="ExternalInput").ap()
    hidt = nc.dram_tensor("hidt", [128, DC, 16], BF16, kind="ExternalInput").ap()
    wet = nc.dram_tensor("wet", [128, DC, H], BF16, kind="ExternalInput").ap()
    wht = nc.dram_tensor("wht", [128, DC, H], BF16, kind="ExternalInput").ap()
    v4 = nc.dram_tensor("v4", [128, HT, 128], BF16, kind="ExternalInput").ap()
    vcol = nc.dram_tensor("vcol", [128, HT], F32, kind="ExternalInput").ap()
    bvec = nc.dram_tensor("bvec", [128, HT], F32, kind="ExternalInput").ap()
    out = nc.dram_tensor("out", [BC, T], F32, kind="ExternalOutput").ap()

    Tanh = mybir.ActivationFunctionType.Tanh
    Exp = mybir.ActivationFunctionType.Exp
    Copy = mybir.ActivationFunctionType.Copy

    with tile.TileContext(nc) as tc:
        with (
            tc.tile_pool(name="singles", bufs=1) as singles,
            tc.tile_pool(name="natf", bufs=5) as natf_pool,
            tc.tile_pool(name="nat", bufs=4) as nat_pool,
            tc.tile_pool(name="encT", bufs=4) as encT_pool,
            tc.tile_pool(name="energy", bufs=3) as en_pool,
            tc.tile_pool(name="stree", bufs=2) as stree_pool,
            tc.tile_pool(name="psh", bufs=1, space="PSUM") as psh_pool,
            tc.tile_pool(name="pss", bufs=1, space="PSUM") as pss_pool,
            tc.tile_pool(name="ptr", bufs=2, space="PSUM") as ptr_pool,
            tc.tile_pool(name="small", bufs=4) as small,
        ):
            NIT = BC * NTT
            # identity for TensorE transposes
            ident = singles.tile([128, 128], BF16)
            from concourse.masks import make_identity
            make_identity(nc, ident)

            # scores for batch b live on partition 32*b (engine ops need
            # 32-aligned start partitions). Partitions != 32*b are never
            # initialized; only partitions 32*b are DMA'd out.
            scores_sb = singles.tile([128, T], F32)
            partials = singles.tile([128, NIT], F32)
            ones_sb = singles.tile([128, 128], BF16)
            nc.gpsimd.memset(partials, 0.0)
            nc.gpsimd.memset(ones_sb, 1.0)

            natf_t = {}
            nat_t = {}
            encT_t = {}

            def emit_load(k):
                # per-q loads: finer DMA-queue granularity and casts can
                # start on the first 512KB
                b, tt = divmod(k, NTT)
                natf = natf_pool.tile([128, 4, D], F32)
                src = enc[b, tt * TT:(tt + 1) * TT, :].rearrange(
                    "(q p) d -> p q d", p=128)
                for q in range(4):
                    nc.sync.dma_start(out=natf[:, q, :], in_=src[:, q, :])
                natf_t[k] = natf

            def emit_cast(k):
                natf = natf_t.pop(k)
                nat = nat_pool.tile([128, 4, D], BF16)
                for q in range(4):
                    if q == 3 and k >= 2:
                        nc.scalar.activation(out=nat[:, q, :],
                                             in_=natf[:, q, :], func=Copy)
                    else:
                        nc.vector.tensor_copy(out=nat[:, q, :],
                                              in_=natf[:, q, :])
                nat_t[k] = nat

            def emit_trans(k, ramp=False):
                # TensorE transposes (67ns each issue-to-issue, LDW hidden)
                # + DVE/ACT psum->sbuf drains. The DMA XBAR path was measured
                # repeatedly (incl. on a dedicated ACT HWDGE queue): the
                # framework fences each xbar transpose against in-flight DMA
                # queue slots, serializing it with the enc load stream.
                nat = nat_t.pop(k)
                encT = encT_pool.tile([128, DC, TT], BF16)
                for q in range(4):
                    pst = ptr_pool.tile([128, DC, 128], BF16)
                    for dc in range(DC):
                        nc.tensor.transpose(
                            pst[:, dc, :],
                            nat[:, q, dc * 128:(dc + 1) * 128],
                            ident,
                        )
                    dst = encT[:, :, q * 128:(q + 1) * 128]
                    if q % 2 == 0 or ramp:
                        nc.vector.tensor_copy(out=dst, in_=pst)
                    else:
                        nc.scalar.activation(out=dst, in_=pst, func=Copy)
                encT_t[k] = encT

            vsum_t = {}
            energy_t = {}

            def emit_mm_tanh(k, last=False):
                b, tt = divmod(k, NTT)
                encT = encT_t.pop(k)
                # energy = tanh(enc @ WeT + c[b]) ; psum [h=128, t=512]
                # ht-inner so consecutive matmuls alternate PSUM banks
                energy = en_pool.tile([128, HT, TT], BF16)
                pshs = [psh_pool.tile([128, TT], F32,
                                      name=f"psh{(HT * k + ht) % 5}")
                        for ht in range(HT)]
                for dc in range(DC):
                    for ht in range(HT):
                        nc.tensor.matmul(
                            pshs[ht],
                            lhsT=wet_sb[:, dc, ht * 128:(ht + 1) * 128],
                            rhs=encT[:, dc, :],
                            start=(dc == 0),
                            stop=(dc == DC - 1),
                        )
                for ht in range(HT):
                    nc.scalar.activation(
                        out=energy[:, ht, :],
                        in_=pshs[ht],
                        func=Tanh,
                        bias=c_sb[:, ht, b:b + 1],
                        scale=1.0,
                    )
                energy_t[k] = energy
                if last:
                    # last tile: emit_scores_mm uses the v4 chain on energy
                    # directly; no stree needed
                    vsum_t[k] = None
                    return
                # scores[t] = sum_h energy[h,t] v[h]: fold v and the 4
                # h-chunks on DVE; the 128-partition sum happens in
                # emit_scores_mm one iteration later (so the PE never waits
                # on this tile's tanh -> stree chain).
                m01 = stree_pool.tile([128, 2, TT], BF16)
                m23 = stree_pool.tile([128, 2, TT], BF16)
                ssum = stree_pool.tile([128, 2, TT], BF16)
                vsum = stree_pool.tile([128, TT], BF16)
                nc.vector.tensor_tensor(
                    m01, energy[:, 0:2, :],
                    vcol_sb[:, 0:2, None].to_broadcast((128, 2, TT)),
                    mybir.AluOpType.mult)
                nc.vector.tensor_tensor(
                    m23, energy[:, 2:4, :],
                    vcol_sb[:, 2:4, None].to_broadcast((128, 2, TT)),
                    mybir.AluOpType.mult)
                nc.vector.tensor_tensor(
                    ssum[:, 0, :], m01[:, 0, :], m01[:, 1, :],
                    mybir.AluOpType.add)
                nc.vector.tensor_tensor(
                    ssum[:, 1, :], m23[:, 0, :], m23[:, 1, :],
                    mybir.AluOpType.add)
                nc.vector.tensor_tensor(
                    vsum, ssum[:, 0, :], ssum[:, 1, :],
                    mybir.AluOpType.add)
                vsum_t[k] = vsum

            def emit_scores_mm(k, last=False):
                b, tt = divmod(k, NTT)
                vsum = vsum_t.pop(k)
                energy = energy_t.pop(k)
                pss = pss_pool.tile([128, TT], F32)
                if last:
                    # v4 chain: starts as soon as tanh(ht0) lands, skipping
                    # the stree latency on the exposed tail
                    for hc in range(HT):
                        nc.tensor.matmul(pss, lhsT=v4_sb[:, hc, :],
                                         rhs=energy[:, hc, :],
                                         start=(hc == 0), stop=(hc == HT - 1))
                else:
                    nc.tensor.matmul(pss, lhsT=ones_sb, rhs=vsum,
                                     start=True, stop=True)
                # all psum partitions now carry the scores row; exp the
                # 32b-aligned one into scores_sb, with the denominator
                # partial accumulating per tile.
                p0 = 32 * b
                nc.scalar.activation(
                    out=scores_sb[p0:p0 + 1, tt * TT:(tt + 1) * TT],
                    in_=pss[p0:p0 + 1, :],
                    func=Exp,
                    scale=1.0,
                    accum_out=partials[p0:p0 + 1, k:k + 1])

            # --- prologue ---
            # sync queue: enc loads only, streaming from t=0
            for k in range(4):
                emit_load(k)

            # ACT HWDGE: wet first (gates the first main matmul); wht and
            # v4 last (hid-projection bias / tail only) so they don't steal
            # ramp HBM bandwidth from the tile-0 loads
            wet_sb = singles.tile([128, DC, H], BF16)
            nc.scalar.dma_start(out=wet_sb, in_=wet)
            hidT = singles.tile([128, DC, 16], BF16)
            nc.scalar.dma_start(out=hidT, in_=hidt)
            b_sb = singles.tile([128, HT], F32)
            nc.scalar.dma_start(out=b_sb, in_=bvec)
            vcol_sb = singles.tile([128, HT], F32)
            nc.scalar.dma_start(out=vcol_sb, in_=vcol)
            wht_sb = singles.tile([128, DC, H], BF16)
            nc.scalar.dma_start(out=wht_sb, in_=wht)
            v4_sb = singles.tile([128, HT, 128], BF16)
            nc.scalar.dma_start(out=v4_sb, in_=v4)

            emit_cast(0)
            emit_trans(0, ramp=True)

            # hidden projection: c[h, b] = hidden[b,:] @ Wh[h,:] + b_attn[h]
            psum_c = pss_pool.tile([128, HT, BC], F32, tag="pss")
            for ht in range(HT):
                for dc in range(DC):
                    nc.tensor.matmul(
                        psum_c[:, ht, :],
                        lhsT=wht_sb[:, dc, ht * 128:(ht + 1) * 128],
                        rhs=hidT[:, dc, :BC],
                        start=(dc == 0),
                        stop=(dc == DC - 1),
                    )
            c_sb = singles.tile([128, HT, BC], F32)
            nc.vector.tensor_tensor(
                c_sb[:],
                psum_c[:],
                b_sb[:, :, None].to_broadcast((128, HT, BC)),
                mybir.AluOpType.add,
            )

            emit_cast(1)

            # --- main loop ---
            # PE order:  trans(k+1) | MMs(k) | ones(k-1) | trans(k+2) | ...
            # DVE order: drains(k+1) | casts(k+2) | stree(k) | ...
            # ACT order: drains(k+1) | tanh(k)x4 | exp(k-1) | ...
            for k in range(NIT):
                if k + 1 < NIT:
                    emit_trans(k + 1)
                if k + 2 < NIT:
                    emit_cast(k + 2)
                emit_mm_tanh(k, last=(k == NIT - 1))
                if k > 0:
                    emit_scores_mm(k - 1)
                if k + 4 < NIT:
                    emit_load(k + 4)
            emit_scores_mm(NIT - 1, last=True)

            # --- tail: softmax normalization for all batches at once ---
            sums = small.tile([128, 1], F32)
            rs = small.tile([128, 1], F32)
            nc.vector.tensor_reduce(sums, partials,
                                    axis=mybir.AxisListType.X,
                                    op=mybir.AluOpType.add)
            nc.vector.reciprocal(rs, sums)
            H2 = T // 2
            nc.vector.tensor_scalar_mul(scores_sb[:, :H2],
                                        scores_sb[:, :H2], rs)
            nc.scalar.activation(out=scores_sb[:, H2:],
                                 in_=scores_sb[:, H2:],
                                 func=Copy, scale=rs)
            sc4 = scores_sb[:, :].rearrange("(g s) t -> g s t", s=32)
            nc.sync.dma_start(out=out[:, :H2], in_=sc4[:, 0, :H2])
            nc.sync.dma_start(out=out[:, H2:], in_=sc4[:, 0, H2:])

    nc.compile()
    return nc


def _prep_shared(W_attn, b_attn, v):
    """Host-side packing of the small replicated parameters."""
    Wh = W_attn[:, :D]                      # [H, D]
    We = W_attn[:, D:]                      # [H, D]
    # wet[p, dc, h] = We[h, dc*128+p]
    wet = np.ascontiguousarray(
        We.T.reshape(DC, 128, H).transpose(1, 0, 2)).astype(ml_dtypes.bfloat16)
    wht = np.ascontiguousarray(
        Wh.T.reshape(DC, 128, H).transpose(1, 0, 2)).astype(ml_dtypes.bfloat16)
    # vcol[p, hc] = v[hc*128+p]
    vcol = np.ascontiguousarray(v.reshape(HT, 128).T).astype(np.float32)
    # v4[p, hc, j] = v[hc*128+p] replicated over 128 stationary columns
    v4 = np.ascontiguousarray(
        np.repeat(v.reshape(HT, 128).T[:, :, None], 128, axis=2)
    ).astype(ml_dtypes.bfloat16)
    bvec = np.ascontiguousarray(b_attn.reshape(HT, 128).T).astype(np.float32)
    return wet, wht, vcol, v4, bvec


def _run(inputs, trace=False):
    hidden = np.asarray(inputs["hidden"], dtype=np.float32)
    enc = np.asarray(inputs["encoder_outputs"], dtype=np.float32)
    W_attn = np.asarray(inputs["W_attn"], dtype=np.float32)
    b_attn = np.asarray(inputs["b_attn"], dtype=np.float32)
    v = np.asarray(inputs["v"], dtype=np.float32)

    wet, wht, vcol, v4, bvec = _prep_shared(W_attn, b_attn, v)

    if "nc" not in _BUILD_CACHE:
        _BUILD_CACHE["nc"] = _build_nc()
    nc = _BUILD_CACHE["nc"]

    in_maps = []
    for i in range(NCORES):
        # hidt[p, dc, j] = hidden[i*BC+j, dc*128+p] (bf16, j padded to 16)
        hcore = hidden[i * BC:(i + 1) * BC]            # [BC, D]
        hidt = np.zeros((128, DC, 16), dtype=ml_dtypes.bfloat16)
        hidt[:, :, :BC] = hcore.T.reshape(DC, 128, BC).transpose(1, 0, 2)
        in_maps.append({
            "enc": enc[i * BC:(i + 1) * BC],
            "hidt": np.ascontiguousarray(hidt),
            "wet": wet,
            "wht": wht,
            "vcol": vcol,
            "v4": v4,
            "bvec": bvec,
        })

    res = run_bass_kernel_spmd(nc, in_maps, core_ids=list(range(NCORES)),
                               trace=trace)
    outs = [np.asarray(res.results[i]["out"], dtype=np.float32)
            for i in range(NCORES)]
    full = np.concatenate(outs, axis=0).reshape(B, 1, T)
    return full, res


def kernel(**inputs) -> np.ndarray:
    out, _ = _run(inputs, trace=False)
    return out


def _ensure_ntff_hook():
    """The trimmed container lacks antenv.axon_hooks; recreate it so
    run_bass_kernel_spmd(trace=True) can drive NTFF profiling via the
    libaxon_pjrt.so C ABI (same as trn_agent_boot._ntff_profile_via_ctypes).
    Only used by the dev/profiling path, never by kernel()."""
    import sys as _sys
    import types
    import ctypes
    import contextlib

    if "antenv.axon_hooks" in _sys.modules:
        return
    so_path = "/opt/axon/libaxon_pjrt.so"
    lib = ctypes.CDLL(so_path)
    if not hasattr(lib, "axon_start_nrt_profile"):
        return
    lib.axon_start_nrt_profile.argtypes = [ctypes.POINTER(ctypes.c_int64),
                                           ctypes.c_size_t]
    lib.axon_start_nrt_profile.restype = ctypes.c_int64
    lib.axon_stop_nrt_profile.argtypes = [ctypes.c_char_p]
    lib.axon_stop_nrt_profile.restype = ctypes.c_int64

    @contextlib.contextmanager
    def _hook(output_dir, device_ids):
        import jax
        jax.devices()
        if device_ids:
            ids = (ctypes.c_int64 * len(device_ids))(*device_ids)
            rc = lib.axon_start_nrt_profile(ids, len(device_ids))
        else:
            rc = lib.axon_start_nrt_profile(None, 0)
        if rc != 0:
            raise RuntimeError(f"axon_start_nrt_profile rc={rc}")
        try:
            yield
        finally:
            n = lib.axon_stop_nrt_profile(str(output_dir).encode())
            print(f"ntff profile: {n} file(s) written to {output_dir}")

    mod = types.ModuleType("antenv.axon_hooks")
    mod.get_axon_ntff_profile_hook = lambda: _hook
    mod.set_axon_ntff_profile_hook = lambda h: None
    _sys.modules["antenv.axon_hooks"] = mod


def kernel_traced(**inputs):
    """Returns (output, exec_time_ns) using the NTFF profile hook."""
    _ensure_ntff_hook()
    out, res = _run(inputs, trace=True)
    return out, res.exec_time_ns
